# revision 6
# baseline (speedup 1.0000x reference)
"""Trainium2 Bass kernel for nn_ConstrainedOutputLayer (Galaxy Zoo 2 style
hierarchical classification head).

Strategy (pure data parallelism over 8 NeuronCores):
  - x [32768, 2048] f32 is split into 8 row shards of 4096. Each shard is
    transposed on the host so the contraction (feature) dim lands on SBUF
    partitions, which lets the tensor engine stream x as the moving operand.
  - The 11 tiny per-class Linear heads are fused into one [37, 2048] weight
    matrix, replicated to every core (pre-swizzled to [128, 16, 37] so each
    128-feature chunk is a ready-to-use stationary operand).
  - Per core: logitsT[37, 4096] = Wcat @ x_c.T via f32r matmuls (full-rate PE,
    ~2^-14 element precision), accumulated over 16 feature chunks into 8 PSUM
    banks of [37, 512].
  - logitsT is copied PSUM->SBUF (fused bias add), PE-transposed back to
    row-major [128, 37] panels, and the sigmoid/softmax hierarchy runs on one
    big [128 rows, 32 groups, 37 classes] SBUF tile with strided APs.
  - Output shard is written as [128, 32, 37] (partition-major, contiguous DMA);
    the host de-interleaves to [4096, 37] and concatenates shards.
"""

import numpy as np

# ---- problem geometry (hardcoded; kernel.py must be self-contained) --------
B, D, C = 32768, 2048, 37
NCORES = 8
RPC = B // NCORES      # rows per core: 4096
KC = D // 128          # 128-feature contraction chunks: 16
NB = RPC // 512        # 512-row PSUM blocks per core: 8
J = RPC // 128         # 128-row groups per core: 32

# (name, start_col, width, parent_col or None) in output-column order.
# parent_col is the *global output column* holding the parent answer prob.
CLASSES = [
    ("class1", 0, 3, None),
    ("class2", 3, 2, 1),      # parent class1.2
    ("class3", 5, 2, 4),      # parent class2.2
    ("class4", 7, 2, 4),      # parent class2.2
    ("class5", 9, 4, 4),      # parent class2.2
    ("class6", 13, 2, None),
    ("class7", 15, 3, 0),     # parent class1.1
    ("class8", 18, 7, 13),    # parent class6.1
    ("class9", 25, 3, 3),     # parent class2.1
    ("class10", 28, 3, 7),    # parent class4.1
    ("class11", 31, 6, 7),    # parent class4.1
]
SOFTMAX = [(n, s, w, p) for (n, s, w, p) in CLASSES if p is not None]  # 9 classes
# order for the final normalize+parent multiply, respecting the dependency
# chain (class2 feeds 3/4/5/9; class4 feeds 10/11)
SM_ORDER = ["class2", "class3", "class4", "class5", "class9",
            "class7", "class8", "class10", "class11"]


def _legalize_waits(nc, mybir):
    """This bass/walrus build allows 1 sem wait per instruction (2 on
    EventSemaphore); Tile's tail drain can exceed that. Split overfull
    on_wait lists into EventSemaphore preludes."""
    n_new = 0
    for f in nc.m.functions:
        for blk in f.blocks:
            out_insts = []
            changed = False
            for inst in blk.instructions:
                si = getattr(inst, "sync_info", None)
                waits = list(si.on_wait) if (si is not None and si.on_wait) else []
                cap = 2 if isinstance(inst, mybir.InstEventSemaphore) else 1
                if len(waits) > cap:
                    excess, keep = waits[:-cap], waits[-cap:]
                    for i in range(0, len(excess), 2):
                        ev = mybir.InstEventSemaphore(
                            name=f"I-waitsplit-{n_new}", ins=[], outs=[])
                        ev.engine = inst.engine
                        ev.sync_info = mybir.SyncInfo(
                            on_wait=excess[i:i + 2], on_update=[])
                        out_insts.append(ev)
                        n_new += 1
                    si.on_wait = keep
                    changed = True
                out_insts.append(inst)
            if changed:
                blk.instructions = out_insts
    return n_new


def _build_program(x_bufs=3):
    import concourse.bass as bass
    import concourse.tile as tile
    from concourse import mybir
    from concourse.masks import make_identity

    f32 = mybir.dt.float32
    f32r = mybir.dt.float32r
    AF = mybir.ActivationFunctionType

    nc = bass.Bass("TRN2", target_bir_lowering=False, debug=False)
    xt = nc.declare_dram_parameter("xt", [D, RPC], f32r, isOutput=False)
    wsb = nc.declare_dram_parameter("wsb", [128, KC, C], f32r, isOutput=False)
    bias = nc.declare_dram_parameter("bias", [C, 1], f32, isOutput=False)
    out = nc.declare_dram_parameter("out", [128, J, C], f32, isOutput=True)

    # xt [2048, 4096] viewed as [feat-in-chunk, chunk, row]
    xt3 = xt.rearrange("(k p) r -> p k r", p=128)

    NH = 2                    # row halves per core
    RH = RPC // NH            # rows per half: 2048
    BH = NB // NH             # 512-row psum blocks per half: 4
    JH = J // NH              # 128-row groups per half: 16

    with tile.TileContext(nc) as tc:
        with (
            tc.tile_pool(name="consts", bufs=1) as consts,
            tc.tile_pool(name="xin", bufs=x_bufs) as xin,
            tc.tile_pool(name="lt", bufs=3) as ltpool,
            tc.tile_pool(name="big", bufs=1) as big,
            tc.tile_pool(name="stats", bufs=2) as stats,
            tc.tile_pool(name="ps", bufs=8, space="PSUM") as psum,
        ):
            # per-chunk x tiles for each row-half: per-partition records are
            # 8 KiB contiguous (2048 rows), near-full DMA rate. Emit the first
            # chunk's DMA before anything else so x streaming starts early.
            xk_tiles = {}
            xk = xin.tile([128, RH], f32r, tag="xk", name="xk_h0_k0")
            nc.sync.dma_start(xk[:, :], xt3[:, 0, 0:RH])
            xk_tiles[(0, 0)] = xk

            # constants (scalar HWDGE ring, off the x-load ring)
            w_t = consts.tile([128, KC, C], f32r, tag="w")
            nc.scalar.dma_start(w_t[:, :, :], wsb[:, :, :])
            b_t = consts.tile([C, 1], f32, tag="b")
            nc.scalar.dma_start(b_t[:, :], bias[:, :])
            ident = consts.tile([128, 128], f32, tag="id")
            make_identity(nc, ident[:, :])

            for h in range(NH):
                for k in range(KC):
                    if (h, k) in xk_tiles:
                        continue
                    xk = xin.tile([128, RH], f32r, tag="xk", name=f"xk_h{h}_k{k}")
                    nc.sync.dma_start(xk[:, :], xt3[:, k, h * RH:(h + 1) * RH])
                    xk_tiles[(h, k)] = xk

            # big output tile, viewed as [rows_in_group, group, class]
            O = big.tile([128, J * C], f32, tag="O")
            O3 = O[:, :].rearrange("p (j c) -> p j c", c=C)
            sm_idx = {n: i for i, (n, _, _, _) in enumerate(SOFTMAX)}
            NSM = len(SOFTMAX)

            for h in range(NH):
                # logitsT[37, 512] x4 for this half, accumulated over chunks
                accs = [psum.tile([C, 512], f32, tag="ps", name=f"acc_h{h}_b{b}")
                        for b in range(BH)]
                for k in range(KC):
                    xk = xk_tiles[(h, k)]
                    for b in range(BH):
                        nc.tensor.matmul(
                            accs[b][:, :],
                            lhsT=w_t[:, k, :],
                            rhs=xk[:, b * 512:(b + 1) * 512],
                            start=(k == 0),
                            stop=(k == KC - 1),
                        )

                # evacuate + bias, transpose back to row-major panels
                for b in range(BH):
                    lt = ltpool.tile([C, 512], f32, tag="lt", name=f"lt{h}_{b}")
                    nc.scalar.activation(lt[:, :], accs[b][:, :], AF.Identity,
                                         bias=b_t[:, 0:1], scale=1.0)
                    for t in range(4):
                        j = h * JH + b * 4 + t
                        tp = psum.tile([128, C], f32, tag="ps", name=f"tp{j}")
                        nc.tensor.transpose(tp[:, :],
                                            lt[:, t * 128:(t + 1) * 128],
                                            ident[:C, :C])
                        nc.vector.tensor_copy(O3[:, j, :], tp[:, :])

                # ---- hierarchy epilogue on this half's O3 [128, 16, 37] ----
                V = O3[:, h * JH:(h + 1) * JH, :]
                nc.scalar.activation(V[:, :, 0:3], V[:, :, 0:3], AF.Sigmoid)
                nc.scalar.activation(V[:, :, 13:15], V[:, :, 13:15], AF.Sigmoid)
                nc.scalar.activation(V[:, :, 3:13], V[:, :, 3:13], AF.Exp)
                nc.scalar.activation(V[:, :, 15:37], V[:, :, 15:37], AF.Exp)

                den = stats.tile([128, NSM, JH], f32, tag="den", name=f"den{h}")
                rec = stats.tile([128, NSM, JH], f32, tag="rec", name=f"rec{h}")
                scl = stats.tile([128, NSM, JH], f32, tag="scl", name=f"scl{h}")
                for i, (_, s, w, _) in enumerate(SOFTMAX):
                    nc.vector.tensor_reduce(
                        out=den[:, i, :], in_=V[:, :, s:s + w],
                        axis=mybir.AxisListType.X, op=mybir.AluOpType.add)
                nc.vector.reciprocal(rec[:, :, :], den[:, :, :])

                for name in SM_ORDER:
                    i = sm_idx[name]
                    _, s, w, p = SOFTMAX[i]
                    nc.vector.tensor_mul(scl[:, i, :], rec[:, i, :], V[:, :, p])
                    nc.vector.tensor_mul(
                        V[:, :, s:s + w], V[:, :, s:s + w],
                        scl[:, i, :][:, :, None].broadcast_to([128, JH, w]))

                # per-half output write on the scalar HWDGE ring
                nc.scalar.dma_start(out[:, h * JH:(h + 1) * JH, :], V)

    _legalize_waits(nc, mybir)
    return nc


_PROGRAM_CACHE = {}


def _get_program():
    if "nc" not in _PROGRAM_CACHE:
        _PROGRAM_CACHE["nc"] = _build_program()
    return _PROGRAM_CACHE["nc"]


def _prep_weights(Ws, bs):
    Wcat = np.concatenate(
        [np.asarray(Ws[name], dtype=np.float32) for name, _, _, _ in CLASSES], axis=0)
    bcat = np.concatenate(
        [np.asarray(bs[name], dtype=np.float32) for name, _, _, _ in CLASSES], axis=0)
    assert Wcat.shape == (C, D) and bcat.shape == (C,)
    # [37, 2048] -> [128 feat-in-chunk, 16 chunk, 37 class]
    wsb = np.ascontiguousarray(Wcat.T.reshape(KC, 128, C).transpose(1, 0, 2))
    return wsb, np.ascontiguousarray(bcat.reshape(C, 1))


def kernel(x, Ws, bs):
    from concourse.bass_utils import run_bass_kernel_spmd

    x = np.asarray(x, dtype=np.float32)
    assert x.shape == (B, D)
    wsb, bias = _prep_weights(Ws, bs)

    nc = _get_program()
    in_maps = []
    for c in range(NCORES):
        xt_c = np.ascontiguousarray(x[c * RPC:(c + 1) * RPC].T)
        in_maps.append({"xt": xt_c, "wsb": wsb, "bias": bias})

    res = run_bass_kernel_spmd(nc, in_maps, list(range(NCORES)))

    out = np.empty((B, C), dtype=np.float32)
    for c in range(NCORES):
        shard = res.results[c]["out"]  # [128, 32, 37]
        out[c * RPC:(c + 1) * RPC] = shard.transpose(1, 0, 2).reshape(RPC, C)
    return out
